# revision 5
# baseline (speedup 1.0000x reference)
"""Single-head causal attention (V=K source bug) on 8 trn2 NeuronCores.

Problem: x[4,2048,1024], W_Q/W_K/W_V[64,1024] (W_V unused by reference).
  Q = x @ W_Q.T ; K = x @ W_K.T ; V = K (reference bug)
  out = softmax(mask(Q K^T / sqrt(1024))) @ V      -> [4,2048,64]

Sharding: 2 cores per batch (core i: batch = i % 4, role r = i // 4).
Each batch's 8 query tiles of 256 rows are split by parity: r=0 gets even
tiles {0,2,4,6}, r=1 odd {1,3,5,7}. Causal prefix for tile q is 2q+2
chunks of 128 keys, so per-core chunk totals are 32 (r=0) and 40 (r=1).
To keep ONE SPMD graph for all 8 cores, each core processes 4 query
"slots" with fixed chunk counts {4,8,12,16} (the r=1 shape); r=0 cores
mask away the over-provisioned chunks via per-core mask data. No
inter-core communication (collective latency floor >> kernel time).

Host-side prep (numpy): transpose x per batch, gather own query columns,
build fused [W_K.T | W_Q.T] weight, build per-core causal masks, and
normalize + transpose the per-core [65,1024] raw output (row 64 is the
softmax denominator from a ones-column in the PV matmul).
"""

import os
import sys

sys.path.insert(0, "/opt/trn_rl_repo")

import numpy as np
import ml_dtypes

BF16 = ml_dtypes.bfloat16

B, T, C, D = 4, 2048, 1024, 64
N_CORES = 8
QTILE = 256          # query rows per slot
N_SLOTS = 4          # slots per core
CHUNK = 128          # key chunk (partition dim of S^T tiles)
SLOT_CHUNKS = [4, 8, 12, 16]   # chunks per slot (uniform graph shape)
GROUP = 4            # chunks per exp/mask group (PSUM group = [128, 4*256])
SCALE = C ** -0.5

# knobs for test harness
TRACE = False
TRACE_CORES = None
LAST_RESULTS = None


def _build_graph():
    import concourse.bass as bass
    import concourse.mybir as mybir
    import concourse.tile as tile
    from concourse import bacc
    from concourse.masks import make_identity

    fp32 = mybir.dt.float32
    bf16 = mybir.dt.bfloat16

    nc = bacc.Bacc(
        "TRN2",
        target_bir_lowering=False,
        debug=False,
        num_devices=N_CORES,
    )

    xkt = nc.dram_tensor("xkt", [C, T], bf16, kind="ExternalInput").ap()
    xqt = nc.dram_tensor("xqt", [C, N_SLOTS * QTILE], bf16, kind="ExternalInput").ap()
    wkq = nc.dram_tensor("wkq", [C, 2 * D], bf16, kind="ExternalInput").ap()
    maskd = nc.dram_tensor(
        "mask", [CHUNK, N_SLOTS * GROUP, QTILE], bf16, kind="ExternalInput"
    ).ap()
    out = nc.dram_tensor("out", [D + 1, N_SLOTS * QTILE], fp32, kind="ExternalOutput").ap()

    NQ = N_SLOTS * QTILE           # 1024 own query rows
    NCH = T // CHUNK               # 16 key chunks total
    CCH = C // CHUNK               # 8 contraction chunks

    from contextlib import ExitStack

    with tile.TileContext(nc) as tc, ExitStack() as ctx:
        consts = ctx.enter_context(tc.tile_pool(name="consts", bufs=1))
        xpool = ctx.enter_context(tc.tile_pool(name="xpool", bufs=1))
        kqpool = ctx.enter_context(tc.tile_pool(name="kqpool", bufs=1))
        ptpool = ctx.enter_context(tc.tile_pool(name="ptpool", bufs=3))
        opool = ctx.enter_context(tc.tile_pool(name="opool", bufs=2))
        psP = ctx.enter_context(tc.tile_pool(name="psP", bufs=2, space="PSUM"))
        psS = ctx.enter_context(tc.tile_pool(name="psS", bufs=2, space="PSUM"))
        psO = ctx.enter_context(tc.tile_pool(name="psO", bufs=2, space="PSUM"))

        # ---- constants / inputs to SBUF ----
        ident = consts.tile([128, 128], bf16)
        make_identity(nc, ident)

        # warm the ACT exp table during the DMA phase
        warm = consts.tile([1, 1], fp32)
        nc.vector.memset(warm, 0.0)
        nc.scalar.activation(warm, warm, mybir.ActivationFunctionType.Exp)

        w_sb = consts.tile([128, CCH, 2 * D], bf16)
        nc.sync.dma_start(
            out=w_sb, in_=wkq.rearrange("(c p) d -> p c d", p=128)
        )
        mask_sb = consts.tile([128, N_SLOTS * GROUP, QTILE], bf16)
        nc.sync.dma_start(out=mask_sb, in_=maskd)

        xkt_sb = xpool.tile([128, CCH, T], bf16)
        nc.sync.dma_start(out=xkt_sb, in_=xkt.rearrange("(c p) t -> p c t", p=128))
        xqt_sb = xpool.tile([128, CCH, NQ], bf16)
        nc.sync.dma_start(out=xqt_sb, in_=xqt.rearrange("(c p) t -> p c t", p=128))

        kT_sb = kqpool.tile([64, T], bf16)        # K^T  [d, s]
        qT_sb = kqpool.tile([64, NQ], bf16)       # Q^T  [d, own t]
        vones = kqpool.tile([128, NCH, D + 1], bf16)  # [V | 1] per key chunk
        nc.vector.memset(vones[:, :, D : D + 1], 1.0)

        # ---- projections ----
        # K^T tile n: [64, 512] = sum_c (w_K chunk).T @ xkt chunk slice
        for n0 in range(0, T // 512, 2):
            ps = [psP.tile([64, 512], fp32, tag="proj", name=f"proj{n0}_{_i}") for _i in range(2)]
            for c in range(CCH):
                for dn in range(2):
                    n = n0 + dn
                    nc.tensor.matmul(
                        ps[dn],
                        lhsT=w_sb[:, c, 0:D],
                        rhs=xkt_sb[:, c, n * 512 : (n + 1) * 512],
                        start=(c == 0),
                        stop=(c == CCH - 1),
                    )
            for dn in range(2):
                n = n0 + dn
                nc.scalar.copy(kT_sb[:, n * 512 : (n + 1) * 512], ps[dn])

        for n0 in range(0, NQ // 512, 2):
            ps = [psP.tile([64, 512], fp32, tag="proj", name=f"proj{n0}_{_i}") for _i in range(2)]
            for c in range(CCH):
                for dn in range(2):
                    n = n0 + dn
                    nc.tensor.matmul(
                        ps[dn],
                        lhsT=w_sb[:, c, D : 2 * D],
                        rhs=xqt_sb[:, c, n * 512 : (n + 1) * 512],
                        start=(c == 0),
                        stop=(c == CCH - 1),
                    )
            for dn in range(2):
                n = n0 + dn
                nc.scalar.copy(qT_sb[:, n * 512 : (n + 1) * 512], ps[dn])

        # ---- K natural ([V | ones]) via PE transpose, 2 chunks per PSUM tile ----
        for k0 in range(0, NCH, 2):
            pt2 = psO.tile([128, 128], bf16, tag="o")
            for dk in range(2):
                k = k0 + dk
                nc.tensor.transpose(
                    pt2[:, dk * 64 : (dk + 1) * 64],
                    in_=kT_sb[:, k * CHUNK : (k + 1) * CHUNK],
                    identity=ident[0:64, 0:64],
                )
            nc.vector.tensor_copy(vones[:, k0 : k0 + 2, 0:D], pt2)

        # ---- attention slots ----
        for j in range(N_SLOTS):
            nch = SLOT_CHUNKS[j]
            o_ps = psO.tile([D + 1, QTILE], fp32, tag="o")
            for g in range(nch // GROUP):
                s_ps = psS.tile([128, GROUP * QTILE], fp32, tag="s")
                for cc in range(GROUP):
                    k = g * GROUP + cc
                    nc.tensor.matmul(
                        s_ps[:, cc * QTILE : (cc + 1) * QTILE],
                        lhsT=kT_sb[:, k * CHUNK : (k + 1) * CHUNK],
                        rhs=qT_sb[:, j * QTILE : (j + 1) * QTILE],
                        start=True,
                        stop=True,
                    )
                pt = ptpool.tile([128, GROUP * QTILE], bf16)
                nc.scalar.activation(
                    pt, s_ps, mybir.ActivationFunctionType.Exp, scale=SCALE
                )
                if g == nch // GROUP - 1:
                    nc.vector.tensor_mul(
                        pt,
                        pt,
                        mask_sb[:, j * GROUP : (j + 1) * GROUP, :].rearrange(
                            "p g q -> p (g q)"
                        ),
                    )
                for cc in range(GROUP):
                    k = g * GROUP + cc
                    nc.tensor.matmul(
                        o_ps,
                        lhsT=vones[:, k, :],
                        rhs=pt[:, cc * QTILE : (cc + 1) * QTILE],
                        start=(k == 0),
                        stop=(k == nch - 1),
                    )
            o_sb = opool.tile([D + 1, QTILE], fp32)
            nc.vector.tensor_copy(o_sb, o_ps)
            nc.sync.dma_start(out=out[:, j * QTILE : (j + 1) * QTILE], in_=o_sb)

    nc.compile()
    return nc


_NC_CACHE = None


def _get_nc():
    global _NC_CACHE
    if _NC_CACHE is None:
        _NC_CACHE = _build_graph()
    return _NC_CACHE


def _host_prep(x, W_Q, W_K):
    """Build per-core input maps."""
    in_maps = []
    wkq = np.concatenate([W_K.T, W_Q.T], axis=1).astype(BF16)  # [1024, 128]
    for i in range(N_CORES):
        b, r = i % B, i // B
        xt = np.ascontiguousarray(x[b].T).astype(BF16)  # [1024, 2048]
        qcols = np.concatenate(
            [
                np.arange(QTILE * (2 * j + r), QTILE * (2 * j + r) + QTILE)
                for j in range(N_SLOTS)
            ]
        )
        xq = np.ascontiguousarray(xt[:, qcols])  # [1024, 1024]
        # mask[p, j*GROUP+cc, f]: key s_abs = 512*j + 128*cc + p,
        # query t_abs = 256*(2j+r) + f ; valid iff s_abs <= t_abs
        p = np.arange(CHUNK)[:, None, None]
        jj = np.arange(N_SLOTS)[None, :, None, None]
        cc = np.arange(GROUP)[None, None, :, None]
        f = np.arange(QTILE)[None, None, None, :]
        s_abs = 512 * jj + CHUNK * cc + np.arange(CHUNK)[:, None, None, None]
        t_abs = QTILE * (2 * jj + r) + f
        m = (s_abs <= t_abs).astype(BF16)  # [128, 4, 4, 256]
        m = m.reshape(CHUNK, N_SLOTS * GROUP, QTILE)
        in_maps.append({"xkt": xt, "xqt": xq, "wkq": wkq, "mask": np.ascontiguousarray(m)})
    return in_maps


def _ensure_ntff_hook():
    """Install the antenv.axon_hooks shim so trace=True works under axon."""
    import types

    try:
        from antenv.axon_hooks import get_axon_ntff_profile_hook  # noqa: F401

        return
    except ImportError:
        pass
    import antenv

    mod = types.ModuleType("antenv.axon_hooks")
    mod._hook = None

    def set_axon_ntff_profile_hook(h):
        mod._hook = h

    def get_axon_ntff_profile_hook():
        return mod._hook

    mod.set_axon_ntff_profile_hook = set_axon_ntff_profile_hook
    mod.get_axon_ntff_profile_hook = get_axon_ntff_profile_hook
    sys.modules["antenv.axon_hooks"] = mod
    antenv.axon_hooks = mod
    try:
        from trn_agent_boot.trn_boot import _ntff_profile_via_ctypes

        hook = _ntff_profile_via_ctypes("/opt/axon/libaxon_pjrt.so")
        if hook is not None:
            set_axon_ntff_profile_hook(hook)
    except Exception as e:  # degrade to no tracing
        print(f"ntff hook install failed: {e}")


def kernel(x, W_Q, W_K, W_V=None, **_unused):
    global LAST_RESULTS
    if TRACE:
        _ensure_ntff_hook()
    x = np.asarray(x, dtype=np.float32)
    W_Q = np.asarray(W_Q, dtype=np.float32)
    W_K = np.asarray(W_K, dtype=np.float32)

    from concourse.bass_utils import run_bass_kernel_spmd

    nc = _get_nc()
    in_maps = _host_prep(x, W_Q, W_K)
    res = run_bass_kernel_spmd(
        nc,
        in_maps,
        core_ids=list(range(N_CORES)),
        trace=TRACE,
        trace_cores=TRACE_CORES,
    )
    LAST_RESULTS = res

    y = np.empty((B, T, D), dtype=np.float32)
    for i in range(N_CORES):
        b, r = i % B, i // B
        ot = res.results[i]["out"]  # [65, 1024]
        o = ot[0:D, :] / ot[D : D + 1, :]
        for j in range(N_SLOTS):
            t0 = QTILE * (2 * j + r)
            y[b, t0 : t0 + QTILE, :] = o[:, j * QTILE : (j + 1) * QTILE].T
    return y


# revision 8
# speedup vs baseline: 1.1930x; 1.1930x over previous
"""Single-head causal attention (V=K source bug) on 8 trn2 NeuronCores.

Problem: x[4,2048,1024], W_Q/W_K/W_V[64,1024] (W_V unused by reference).
  Q = x @ W_Q.T ; K = x @ W_K.T ; V = K (reference bug)
  out = softmax(mask(Q K^T / sqrt(1024))) @ V      -> [4,2048,64]

Sharding: 2 cores per batch (core i: batch = i % 4, role r = i // 4).
Each batch's 8 query tiles of 256 rows split by parity (r=0 even, r=1 odd).
ONE SPMD graph for all 8 cores. Per-core differences are folded into DATA:

 * x^T is sent column-PERMUTED, own query tiles first:
     positions 0..3 = own tiles (2j+r), positions 4..7 = other tiles.
   So the Q projection reads compile-time columns [0,1024); causality over
   the permuted key order is encoded in per-core 0/1 masks.
 * slot j (own tile 2j+r, query rows 256 of it) attends own chunks
   [0..2j+1] and other chunks [8..8+2j+1] (uniform r=1 shape; r=0 masks
   the over-provisioned tail) -> 4j+4 key chunks of 128.

Device pipeline: warmup matmuls (HAM) -> col-paired projections
(Q pair, K stack A = cols 0-511|1024-1535, K stack B) -> PE transposes for
V natural -> per-slot: row-packed S^T pairs (own chunk on array rows 0-63,
other chunk on rows 64-127, concurrent), exp on ACT (scale folded), mask
mul on the final group, PV matmul with a ones-column producing the softmax
denominator in row 64. Host normalizes + transposes the [65,1024] output.
No collectives (latency floor >> kernel time).
"""

import os
import sys

sys.path.insert(0, "/opt/trn_rl_repo")

import numpy as np
import ml_dtypes

BF16 = ml_dtypes.bfloat16

B, T, C, D = 4, 2048, 1024, 64
N_CORES = 8
QTILE = 256          # query rows per slot
N_SLOTS = 4
CHUNK = 128          # key chunk
GROUP = 4            # chunks per exp group ([128, 4*256] psum tile)
SCALE = C ** -0.5
N_WARMUP = 40        # HAM warmup matmuls

TRACE = False
TRACE_CORES = None
LAST_RESULTS = None


def _slot_chunks(j):
    """Chunk list for slot j, interleaved [own_a, own_a+1, oth_a, oth_a+1]
    per group so that pair (own_a, oth_a) row-packs and the masked set
    {own 2j, own 2j+1, oth 2j, oth 2j+1} is exactly the last group."""
    chunks = []
    for a in range(0, 2 * j + 2, 2):
        chunks += [a, a + 1, 8 + a, 8 + a + 1]
    return chunks


def _chunk_stack(c):
    """abs permuted chunk c -> (stack_idx, half, within). Stack A covers
    permuted cols 0-511 (top) and 1024-1535 (bottom); B covers 512-1023
    (top) and 1536-2047 (bottom)."""
    pos = c // 2            # 256-col tile position 0..7
    if pos < 4:             # own side -> top halves
        return (pos // 2, 0, c % 4)
    else:                   # other side -> bottom halves
        return ((pos - 4) // 2, 1, c % 4)


def _build_graph():
    import concourse.bass as bass
    import concourse.mybir as mybir
    import concourse.tile as tile
    from concourse import bacc
    from concourse.masks import make_identity
    from contextlib import ExitStack

    fp32 = mybir.dt.float32
    bf16 = mybir.dt.bfloat16

    nc = bacc.Bacc(
        "TRN2",
        target_bir_lowering=False,
        debug=False,
        num_devices=N_CORES,
    )

    xkt = nc.dram_tensor("xkt", [C, T], bf16, kind="ExternalInput").ap()
    wkq = nc.dram_tensor("wkq", [C, 2 * D], bf16, kind="ExternalInput").ap()
    maskd = nc.dram_tensor(
        "mask", [CHUNK, N_SLOTS * GROUP, QTILE], bf16, kind="ExternalInput"
    ).ap()
    out = nc.dram_tensor(
        "out", [D + 1, N_SLOTS * QTILE], fp32, kind="ExternalOutput"
    ).ap()

    NQ = N_SLOTS * QTILE           # 1024 own query cols
    NCH = T // CHUNK               # 16 key chunks
    CCH = C // CHUNK               # 8 contraction chunks

    with tile.TileContext(nc) as tc, ExitStack() as ctx:
        consts = ctx.enter_context(tc.tile_pool(name="consts", bufs=1))
        xpool = ctx.enter_context(tc.tile_pool(name="xpool", bufs=1))
        kqpool = ctx.enter_context(tc.tile_pool(name="kqpool", bufs=1))
        ptpool = ctx.enter_context(tc.tile_pool(name="ptpool", bufs=3))
        opool = ctx.enter_context(tc.tile_pool(name="opool", bufs=2))
        psP = ctx.enter_context(tc.tile_pool(name="psP", bufs=2, space="PSUM"))
        psS = ctx.enter_context(tc.tile_pool(name="psS", bufs=2, space="PSUM"))
        psO = ctx.enter_context(tc.tile_pool(name="psO", bufs=2, space="PSUM"))

        # ---- constants ----
        ident = consts.tile([128, 128], bf16)
        make_identity(nc, ident)
        warm_ps = psP.tile([128, 128], fp32, tag="proj")
        for w in range(N_WARMUP):
            nc.tensor.matmul(warm_ps, lhsT=ident, rhs=ident, start=True, stop=True)
        warm = consts.tile([1, 1], fp32)
        nc.vector.memset(warm, 0.0)
        nc.scalar.activation(warm, warm, mybir.ActivationFunctionType.Exp)

        # ---- DMAs (slab order drives the pipeline) ----
        w_sb = consts.tile([128, CCH, 2 * D], bf16)
        nc.sync.dma_start(out=w_sb, in_=wkq.rearrange("(c p) d -> p c d", p=128))
        # xkt slabs: 4 x [128, CCH, 512] column slabs of the permuted x^T
        xs = []
        for s in range(4):
            xsl = xpool.tile([128, CCH, 512], bf16, name=f"xslab{s}")
            nc.sync.dma_start(
                out=xsl,
                in_=xkt.rearrange("(c p) t -> p c t", p=128)[
                    :, :, s * 512 : (s + 1) * 512
                ],
            )
            xs.append(xsl)
            if s == 2:
                mask_sb = consts.tile([128, N_SLOTS * GROUP, QTILE], bf16)
                nc.sync.dma_start(out=mask_sb, in_=maskd)

        # ---- Q projection (col-paired: slabs 0,1 -> psum halves) ----
        qT = kqpool.tile([128, NQ], bf16)   # Q^T duplicated in both halves
        q_ps = psP.tile([128, 512], fp32, tag="proj")
        for c in range(CCH):
            nc.tensor.matmul(
                q_ps[0:64, :], lhsT=w_sb[:, c, D : 2 * D], rhs=xs[0][:, c, :],
                start=(c == 0), stop=(c == CCH - 1),
            )
            nc.tensor.matmul(
                q_ps[64:128, :], lhsT=w_sb[:, c, D : 2 * D], rhs=xs[1][:, c, :],
                start=(c == 0), stop=(c == CCH - 1),
            )
        nc.scalar.copy(qT[0:64, 0:512], q_ps[0:64, :])
        nc.scalar.copy(qT[0:64, 512:1024], q_ps[64:128, :])
        # duplicate into partitions 64-127 (cross-partition -> DMA)
        nc.sync.dma_start(out=qT[64:128, :], in_=qT[0:64, :])

        # ---- K projection stacks + transposes + attention slots ----
        # stack A: top = permuted cols 0-511 (chunks 0-3),
        #          bottom = cols 1024-1535 (chunks 8-11)   [slabs 0, 2]
        # stack B: top = 512-1023 (4-7), bottom = 1536-2047 (12-15) [1, 3]
        kstk = []
        vones = []
        o_done = []

        def kproj(stack_idx):
            slabs = (xs[0], xs[2]) if stack_idx == 0 else (xs[1], xs[3])
            kt = kqpool.tile([128, 512], bf16, name=f"kstk{stack_idx}")
            k_ps = psP.tile([128, 512], fp32, tag="proj", name=f"kps{stack_idx}")
            for c in range(CCH):
                nc.tensor.matmul(
                    k_ps[0:64, :], lhsT=w_sb[:, c, 0:D], rhs=slabs[0][:, c, :],
                    start=(c == 0), stop=(c == CCH - 1),
                )
                nc.tensor.matmul(
                    k_ps[64:128, :], lhsT=w_sb[:, c, 0:D], rhs=slabs[1][:, c, :],
                    start=(c == 0), stop=(c == CCH - 1),
                )
            nc.scalar.copy(kt, k_ps)
            kstk.append(kt)
            # transposes: V natural for this stack's 8 chunks
            vo = kqpool.tile([128, 8, D + 1], bf16, name=f"vones{stack_idx}")
            nc.vector.memset(vo[:, :, D : D + 1], 1.0)
            for p0 in range(4):  # pairs of chunks -> one [128,128] psum tile
                pt2 = psO.tile([128, 128], bf16, tag="o", name=f"tp{stack_idx}_{p0}")
                for dk in range(2):
                    w = p0 * 2 + dk          # within-stack chunk 0..7
                    half, within = w // 4, w % 4
                    nc.tensor.transpose(
                        pt2[:, dk * 64 : (dk + 1) * 64],
                        in_=kt[64 * half : 64 * half + 64,
                               within * CHUNK : (within + 1) * CHUNK],
                        identity=ident[64 * half : 64 * half + 64,
                                       64 * half : 64 * half + 64],
                    )
                nc.vector.tensor_copy(vo[:, p0 * 2 : p0 * 2 + 2, 0:D], pt2)
            vones.append(vo)

        def lhsT_of(c):
            si, half, within = _chunk_stack(c)
            return kstk[si][64 * half : 64 * half + 64,
                            within * CHUNK : (within + 1) * CHUNK]

        def vones_of(c):
            si, half, within = _chunk_stack(c)
            return vones[si][:, half * 4 + within, :]

        def slot(j):
            chunks = _slot_chunks(j)          # 4j+4 chunks, groups of 4
            o_ps = psO.tile([D + 1, QTILE], fp32, tag="o", name=f"ops{j}")
            ngroups = len(chunks) // GROUP
            for g in range(ngroups):
                gch = chunks[g * GROUP : (g + 1) * GROUP]
                s_ps = psS.tile([128, GROUP * QTILE], fp32, tag="s",
                                name=f"sps{j}_{g}")
                # row-packed pairs: own chunk (slice m, rows 0-63) runs
                # concurrently with other chunk (slice m+2, rows 64-127)
                for m in range(2):
                    for sl in (m, m + 2):
                        cc = gch[sl]
                        half = _chunk_stack(cc)[1]
                        nc.tensor.matmul(
                            s_ps[:, sl * QTILE : (sl + 1) * QTILE],
                            lhsT=lhsT_of(cc),
                            rhs=qT[64 * half : 64 * half + 64,
                                   j * QTILE : (j + 1) * QTILE],
                            start=True, stop=True,
                        )
                pt = ptpool.tile([128, GROUP * QTILE], bf16, name=f"pt{j}_{g}")
                nc.scalar.activation(
                    pt, s_ps, mybir.ActivationFunctionType.Exp, scale=SCALE
                )
                if g == ngroups - 1:
                    nc.vector.tensor_mul(
                        pt, pt,
                        mask_sb[:, j * GROUP : (j + 1) * GROUP, :].rearrange(
                            "p g q -> p (g q)"
                        ),
                    )
                for sl, cc in enumerate(gch):
                    k_abs = g * GROUP + sl
                    nc.tensor.matmul(
                        o_ps, lhsT=vones_of(cc),
                        rhs=pt[:, sl * QTILE : (sl + 1) * QTILE],
                        start=(k_abs == 0), stop=(k_abs == len(chunks) - 1),
                    )
            o_sb = opool.tile([D + 1, QTILE], fp32, name=f"osb{j}")
            nc.vector.tensor_copy(o_sb, o_ps)
            nc.sync.dma_start(out=out[:, j * QTILE : (j + 1) * QTILE], in_=o_sb)

        kproj(0)      # stack A (slabs 0, 2)
        slot(0)       # needs chunks {0,1,8,9} in A
        slot(1)       # needs {0..3, 8..11} in A
        kproj(1)      # stack B (slabs 1, 3)
        slot(2)
        slot(3)

    nc.compile()
    return nc


_NC_CACHE = None


def _get_nc():
    global _NC_CACHE
    if _NC_CACHE is None:
        _NC_CACHE = _build_graph()
    return _NC_CACHE


def _perm_tiles(r):
    """permuted 256-col tile order: own tiles (2j+r) first, then others."""
    own = [2 * j + r for j in range(N_SLOTS)]
    oth = [2 * j + (1 - r) for j in range(N_SLOTS)]
    return own + oth


def _host_prep(x, W_Q, W_K):
    in_maps = []
    wkq = np.concatenate([W_K.T, W_Q.T], axis=1).astype(BF16)  # [1024, 128]
    pchunk = np.arange(CHUNK)
    f = np.arange(QTILE)
    for i in range(N_CORES):
        b, r = i % B, i // B
        perm = _perm_tiles(r)
        xt = x[b].T.astype(BF16)  # [1024, 2048]
        cols = np.concatenate(
            [np.arange(QTILE * p, QTILE * p + QTILE) for p in perm]
        )
        xkt = np.ascontiguousarray(xt[:, cols])
        # mask[p, j*GROUP + sl, f] for the LAST group of slot j, chunk list
        # order [own 2j, oth 2j, own 2j+1, oth 2j+1]
        m = np.zeros((CHUNK, N_SLOTS * GROUP, QTILE), dtype=np.float32)
        for j in range(N_SLOTS):
            t_abs = QTILE * (2 * j + r) + f[None, :]
            gch = _slot_chunks(j)[-GROUP:]
            for sl, c in enumerate(gch):
                pos = c // 2
                op = perm[pos]               # original 256-tile index
                s_abs = QTILE * op + CHUNK * (c % 2) + pchunk[:, None]
                m[:, j * GROUP + sl, :] = (s_abs <= t_abs)
        in_maps.append(
            {"xkt": xkt, "wkq": wkq, "mask": np.ascontiguousarray(m.astype(BF16))}
        )
    return in_maps


def _ensure_ntff_hook():
    """Install the antenv.axon_hooks shim so trace=True works under axon."""
    import types

    try:
        from antenv.axon_hooks import get_axon_ntff_profile_hook  # noqa: F401

        return
    except ImportError:
        pass
    import antenv

    mod = types.ModuleType("antenv.axon_hooks")
    mod._hook = None

    def set_axon_ntff_profile_hook(h):
        mod._hook = h

    def get_axon_ntff_profile_hook():
        return mod._hook

    mod.set_axon_ntff_profile_hook = set_axon_ntff_profile_hook
    mod.get_axon_ntff_profile_hook = get_axon_ntff_profile_hook
    sys.modules["antenv.axon_hooks"] = mod
    antenv.axon_hooks = mod
    try:
        from trn_agent_boot.trn_boot import _ntff_profile_via_ctypes

        hook = _ntff_profile_via_ctypes("/opt/axon/libaxon_pjrt.so")
        if hook is not None:
            set_axon_ntff_profile_hook(hook)
    except Exception as e:  # degrade to no tracing
        print(f"ntff hook install failed: {e}")


def kernel(x, W_Q, W_K, W_V=None, **_unused):
    global LAST_RESULTS
    if TRACE:
        _ensure_ntff_hook()
    x = np.asarray(x, dtype=np.float32)
    W_Q = np.asarray(W_Q, dtype=np.float32)
    W_K = np.asarray(W_K, dtype=np.float32)

    from concourse.bass_utils import run_bass_kernel_spmd

    nc = _get_nc()
    in_maps = _host_prep(x, W_Q, W_K)
    res = run_bass_kernel_spmd(
        nc,
        in_maps,
        core_ids=list(range(N_CORES)),
        trace=TRACE,
        trace_cores=TRACE_CORES,
    )
    LAST_RESULTS = res

    y = np.empty((B, T, D), dtype=np.float32)
    for i in range(N_CORES):
        b, r = i % B, i // B
        ot = res.results[i]["out"]  # [65, 1024]
        o = ot[0:D, :] / ot[D : D + 1, :]
        for j in range(N_SLOTS):
            t0 = QTILE * (2 * j + r)
            y[b, t0 : t0 + QTILE, :] = o[:, j * QTILE : (j + 1) * QTILE].T
    return y


# revision 12
# speedup vs baseline: 1.3617x; 1.1414x over previous
"""Single-head causal attention (V=K source bug) on 8 trn2 NeuronCores.

Problem: x[4,2048,1024], W_Q/W_K/W_V[64,1024] (W_V unused by reference).
  Q = x @ W_Q.T ; K = x @ W_K.T ; V = K (reference bug)
  out = softmax(mask(Q K^T / sqrt(1024))) @ V      -> [4,2048,64]

Sharding: 2 cores per batch (core i: batch = i % 4, role r = i // 4).
Each batch's 8 query tiles of 256 rows split by parity (r=0 even, r=1 odd).
ONE SPMD graph for all 8 cores. Per-core differences are folded into DATA:

 * x^T is sent column-PERMUTED, own query tiles first:
     positions 0..3 = own tiles (2j+r), positions 4..7 = other tiles.
   So the Q projection reads compile-time columns [0,1024); causality over
   the permuted key order is encoded in per-core 0/1 masks.
 * slot j (own tile 2j+r, query rows 256 of it) attends own chunks
   [0..2j+1] and other chunks [8..8+2j+1] (uniform r=1 shape; r=0 masks
   the over-provisioned tail) -> 4j+4 key chunks of 128.

Device pipeline: warmup matmuls (HAM) -> col-paired projections
(Q pair, K stack A = cols 0-511|1024-1535, K stack B) -> PE transposes for
V natural -> per-slot: row-packed S^T pairs (own chunk on array rows 0-63,
other chunk on rows 64-127, concurrent), exp on ACT (scale folded), mask
mul on the final group, PV matmul with a ones-column producing the softmax
denominator in row 64. Host normalizes + transposes the [65,1024] output.
No collectives (latency floor >> kernel time).
"""

import os
import sys

sys.path.insert(0, "/opt/trn_rl_repo")

import numpy as np
import ml_dtypes

BF16 = ml_dtypes.bfloat16

B, T, C, D = 4, 2048, 1024, 64
N_CORES = 8
QTILE = 256          # query rows per slot
N_SLOTS = 4
CHUNK = 128          # key chunk
GROUP = 4            # chunks per exp group ([128, 4*256] psum tile)
SCALE = C ** -0.5
N_WARMUP = 40        # HAM warmup matmuls

TRACE = False
TRACE_CORES = None
LAST_RESULTS = None


def _slot_chunks(j):
    """Chunk list for slot j, interleaved [own_a, own_a+1, oth_a, oth_a+1]
    per group so that pair (own_a, oth_a) row-packs and the masked set
    {own 2j, own 2j+1, oth 2j, oth 2j+1} is exactly the last group."""
    chunks = []
    for a in range(0, 2 * j + 2, 2):
        chunks += [a, a + 1, 8 + a, 8 + a + 1]
    return chunks


def _chunk_stack(c):
    """abs permuted chunk c -> (stack_idx, half, within). Stack A covers
    permuted cols 0-511 (top) and 1024-1535 (bottom); B covers 512-1023
    (top) and 1536-2047 (bottom)."""
    pos = c // 2            # 256-col tile position 0..7
    if pos < 4:             # own side -> top halves
        return (pos // 2, 0, c % 4)
    else:                   # other side -> bottom halves
        return ((pos - 4) // 2, 1, c % 4)


def _build_graph():
    import concourse.bass as bass
    import concourse.mybir as mybir
    import concourse.tile as tile
    from concourse import bacc
    from concourse.masks import make_identity
    from contextlib import ExitStack

    fp32 = mybir.dt.float32
    bf16 = mybir.dt.bfloat16

    nc = bacc.Bacc(
        "TRN2",
        target_bir_lowering=False,
        debug=False,
        num_devices=N_CORES,
    )

    xkt = nc.dram_tensor("xkt", [C, T], bf16, kind="ExternalInput").ap()
    wkq = nc.dram_tensor("wkq", [C, 2 * D], bf16, kind="ExternalInput").ap()
    maskd = nc.dram_tensor(
        "mask", [CHUNK, N_SLOTS * GROUP, QTILE], bf16, kind="ExternalInput"
    ).ap()
    out = nc.dram_tensor(
        "out", [D + 1, N_SLOTS * QTILE], fp32, kind="ExternalOutput"
    ).ap()

    NQ = N_SLOTS * QTILE           # 1024 own query cols
    NCH = T // CHUNK               # 16 key chunks
    CCH = C // CHUNK               # 8 contraction chunks

    with tile.TileContext(nc) as tc, ExitStack() as ctx:
        consts = ctx.enter_context(tc.tile_pool(name="consts", bufs=1))
        xpool = ctx.enter_context(tc.tile_pool(name="xpool", bufs=1))
        kqpool = ctx.enter_context(tc.tile_pool(name="kqpool", bufs=1))
        ptpool = ctx.enter_context(tc.tile_pool(name="ptpool", bufs=3))
        opool = ctx.enter_context(tc.tile_pool(name="opool", bufs=2))
        psP = ctx.enter_context(tc.tile_pool(name="psP", bufs=2, space="PSUM"))
        psS = ctx.enter_context(tc.tile_pool(name="psS", bufs=2, space="PSUM"))
        psO = ctx.enter_context(tc.tile_pool(name="psO", bufs=2, space="PSUM"))

        # ---- constants ----
        ident = consts.tile([128, 128], bf16)
        make_identity(nc, ident)
        # one PSUM accumulation group -> back-to-back PE issue, no sem chain
        warm_ps = psP.tile([128, 128], fp32, tag="proj")
        for w in range(N_WARMUP):
            nc.tensor.matmul(
                warm_ps, lhsT=ident, rhs=ident,
                start=(w == 0), stop=(w == N_WARMUP - 1),
            )
        warm = consts.tile([1, 1], fp32)
        nc.vector.memset(warm, 0.0)
        nc.scalar.activation(warm, warm, mybir.ActivationFunctionType.Exp)

        # ---- DMAs (slab order drives the pipeline) ----
        w_sb = consts.tile([128, CCH, 2 * D], bf16)
        nc.sync.dma_start(out=w_sb, in_=wkq.rearrange("(c p) d -> p c d", p=128))
        # xkt slabs: 4 x [128, CCH, 512] column slabs of the permuted x^T
        xs = []
        xkt_r = xkt.rearrange("(c p) t -> p c t", p=128)
        for s in range(4):
            xsl = xpool.tile([128, CCH, 512], bf16, name=f"xslab{s}")
            for c0 in range(0, CCH, 2):  # sub-split so proj starts early
                nc.sync.dma_start(
                    out=xsl[:, c0 : c0 + 2, :],
                    in_=xkt_r[:, c0 : c0 + 2, s * 512 : (s + 1) * 512],
                )
            xs.append(xsl)
            if s == 2:
                mask_sb = consts.tile([128, N_SLOTS * GROUP, QTILE], bf16)
                nc.sync.dma_start(out=mask_sb, in_=maskd)

        # ---- Q projection (col-paired: slabs 0,1 -> psum halves) ----
        qT = kqpool.tile([128, NQ], bf16)   # Q^T duplicated in both halves
        q_ps = psP.tile([128, 512], fp32, tag="proj")
        for c in range(CCH):
            nc.tensor.matmul(
                q_ps[0:64, :], lhsT=w_sb[:, c, D : 2 * D], rhs=xs[0][:, c, :],
                start=(c == 0), stop=(c == CCH - 1),
            )
            nc.tensor.matmul(
                q_ps[64:128, :], lhsT=w_sb[:, c, D : 2 * D], rhs=xs[1][:, c, :],
                start=(c == 0), stop=(c == CCH - 1),
            )
        nc.vector.tensor_copy(qT[0:64, 0:512], q_ps[0:64, :])
        nc.vector.tensor_copy(qT[0:64, 512:1024], q_ps[64:128, :])
        # duplicate into partitions 64-127 (cross-partition -> DMA)
        nc.sync.dma_start(out=qT[64:128, :], in_=qT[0:64, :])

        # ---- K projection stacks + transposes + attention slots ----
        # stack A: top = permuted cols 0-511 (chunks 0-3),
        #          bottom = cols 1024-1535 (chunks 8-11)   [slabs 0, 2]
        # stack B: top = 512-1023 (4-7), bottom = 1536-2047 (12-15) [1, 3]
        kstk = []
        vones = []
        o_done = []

        def kproj(stack_idx):
            slabs = (xs[0], xs[2]) if stack_idx == 0 else (xs[1], xs[3])
            kt = kqpool.tile([128, 512], bf16, name=f"kstk{stack_idx}")
            k_ps = psP.tile([128, 512], fp32, tag="proj", name=f"kps{stack_idx}")
            for c in range(CCH):
                nc.tensor.matmul(
                    k_ps[0:64, :], lhsT=w_sb[:, c, 0:D], rhs=slabs[0][:, c, :],
                    start=(c == 0), stop=(c == CCH - 1),
                )
                nc.tensor.matmul(
                    k_ps[64:128, :], lhsT=w_sb[:, c, 0:D], rhs=slabs[1][:, c, :],
                    start=(c == 0), stop=(c == CCH - 1),
                )
            nc.vector.tensor_copy(kt, k_ps)
            kstk.append(kt)
            # transposes: V natural for this stack's 8 chunks
            vo = kqpool.tile([128, 8, D + 1], bf16, name=f"vones{stack_idx}")
            nc.vector.memset(vo[:, :, D : D + 1], 1.0)
            for p0 in range(4):  # pairs of chunks -> one [128,128] psum tile
                pt2 = psO.tile([128, 128], bf16, tag="o", name=f"tp{stack_idx}_{p0}")
                for dk in range(2):
                    w = p0 * 2 + dk          # within-stack chunk 0..7
                    half, within = w // 4, w % 4
                    nc.tensor.transpose(
                        pt2[:, dk * 64 : (dk + 1) * 64],
                        in_=kt[64 * half : 64 * half + 64,
                               within * CHUNK : (within + 1) * CHUNK],
                        identity=ident[64 * half : 64 * half + 64,
                                       64 * half : 64 * half + 64],
                    )
                nc.vector.tensor_copy(vo[:, p0 * 2 : p0 * 2 + 2, 0:D], pt2)
            vones.append(vo)

        def lhsT_of(c):
            si, half, within = _chunk_stack(c)
            return kstk[si][64 * half : 64 * half + 64,
                            within * CHUNK : (within + 1) * CHUNK]

        def vones_of(c):
            si, half, within = _chunk_stack(c)
            return vones[si][:, half * 4 + within, :]

        def slot(j):
            chunks = _slot_chunks(j)          # 4j+4 chunks, groups of 4
            o_ps = psO.tile([D + 1, QTILE], fp32, tag="o", name=f"ops{j}")
            ngroups = len(chunks) // GROUP
            for g in range(ngroups):
                gch = chunks[g * GROUP : (g + 1) * GROUP]
                s_ps = psS.tile([128, GROUP * QTILE], fp32, tag="s",
                                name=f"sps{j}_{g}")
                # row-packed pairs: own chunk (slice m, rows 0-63) runs
                # concurrently with other chunk (slice m+2, rows 64-127)
                for m in range(2):
                    for sl in (m, m + 2):
                        cc = gch[sl]
                        half = _chunk_stack(cc)[1]
                        nc.tensor.matmul(
                            s_ps[:, sl * QTILE : (sl + 1) * QTILE],
                            lhsT=lhsT_of(cc),
                            rhs=qT[64 * half : 64 * half + 64,
                                   j * QTILE : (j + 1) * QTILE],
                            start=True, stop=True,
                        )
                pt = ptpool.tile([128, GROUP * QTILE], bf16, name=f"pt{j}_{g}")
                nc.scalar.activation(
                    pt, s_ps, mybir.ActivationFunctionType.Exp, scale=SCALE
                )
                if g == ngroups - 1:
                    nc.vector.tensor_mul(
                        pt, pt,
                        mask_sb[:, j * GROUP : (j + 1) * GROUP, :].rearrange(
                            "p g q -> p (g q)"
                        ),
                    )
                for sl, cc in enumerate(gch):
                    k_abs = g * GROUP + sl
                    nc.tensor.matmul(
                        o_ps, lhsT=vones_of(cc),
                        rhs=pt[:, sl * QTILE : (sl + 1) * QTILE],
                        start=(k_abs == 0), stop=(k_abs == len(chunks) - 1),
                    )
            o_sb = opool.tile([D + 1, QTILE], fp32, name=f"osb{j}")
            nc.vector.tensor_copy(o_sb, o_ps)
            nc.sync.dma_start(out=out[:, j * QTILE : (j + 1) * QTILE], in_=o_sb)

        kproj(0)      # stack A (slabs 0, 2)
        slot(0)       # needs chunks {0,1,8,9} in A
        slot(1)       # needs {0..3, 8..11} in A
        kproj(1)      # stack B (slabs 1, 3)
        slot(2)
        slot(3)

    nc.compile()
    return nc


_NC_CACHE = None


def _get_nc():
    global _NC_CACHE
    if _NC_CACHE is None:
        _NC_CACHE = _build_graph()
    return _NC_CACHE


def _perm_tiles(r):
    """permuted 256-col tile order: own tiles (2j+r) first, then others."""
    own = [2 * j + r for j in range(N_SLOTS)]
    oth = [2 * j + (1 - r) for j in range(N_SLOTS)]
    return own + oth


def _host_prep(x, W_Q, W_K):
    in_maps = []
    wkq = np.concatenate([W_K.T, W_Q.T], axis=1).astype(BF16)  # [1024, 128]
    pchunk = np.arange(CHUNK)
    f = np.arange(QTILE)
    for i in range(N_CORES):
        b, r = i % B, i // B
        perm = _perm_tiles(r)
        xt = x[b].T.astype(BF16)  # [1024, 2048]
        cols = np.concatenate(
            [np.arange(QTILE * p, QTILE * p + QTILE) for p in perm]
        )
        xkt = np.ascontiguousarray(xt[:, cols])
        # mask[p, j*GROUP + sl, f] for the LAST group of slot j, chunk list
        # order [own 2j, oth 2j, own 2j+1, oth 2j+1]
        m = np.zeros((CHUNK, N_SLOTS * GROUP, QTILE), dtype=np.float32)
        for j in range(N_SLOTS):
            t_abs = QTILE * (2 * j + r) + f[None, :]
            gch = _slot_chunks(j)[-GROUP:]
            for sl, c in enumerate(gch):
                pos = c // 2
                op = perm[pos]               # original 256-tile index
                s_abs = QTILE * op + CHUNK * (c % 2) + pchunk[:, None]
                m[:, j * GROUP + sl, :] = (s_abs <= t_abs)
        in_maps.append(
            {"xkt": xkt, "wkq": wkq, "mask": np.ascontiguousarray(m.astype(BF16))}
        )
    return in_maps


def _ensure_ntff_hook():
    """Install the antenv.axon_hooks shim so trace=True works under axon."""
    import types

    try:
        from antenv.axon_hooks import get_axon_ntff_profile_hook  # noqa: F401

        return
    except ImportError:
        pass
    import antenv

    mod = types.ModuleType("antenv.axon_hooks")
    mod._hook = None

    def set_axon_ntff_profile_hook(h):
        mod._hook = h

    def get_axon_ntff_profile_hook():
        return mod._hook

    mod.set_axon_ntff_profile_hook = set_axon_ntff_profile_hook
    mod.get_axon_ntff_profile_hook = get_axon_ntff_profile_hook
    sys.modules["antenv.axon_hooks"] = mod
    antenv.axon_hooks = mod
    try:
        from trn_agent_boot.trn_boot import _ntff_profile_via_ctypes

        hook = _ntff_profile_via_ctypes("/opt/axon/libaxon_pjrt.so")
        if hook is not None:
            set_axon_ntff_profile_hook(hook)
    except Exception as e:  # degrade to no tracing
        print(f"ntff hook install failed: {e}")


def kernel(x, W_Q, W_K, W_V=None, **_unused):
    global LAST_RESULTS
    if TRACE:
        _ensure_ntff_hook()
    x = np.asarray(x, dtype=np.float32)
    W_Q = np.asarray(W_Q, dtype=np.float32)
    W_K = np.asarray(W_K, dtype=np.float32)

    from concourse.bass_utils import run_bass_kernel_spmd

    nc = _get_nc()
    in_maps = _host_prep(x, W_Q, W_K)
    res = run_bass_kernel_spmd(
        nc,
        in_maps,
        core_ids=list(range(N_CORES)),
        trace=TRACE,
        trace_cores=TRACE_CORES,
    )
    LAST_RESULTS = res

    y = np.empty((B, T, D), dtype=np.float32)
    for i in range(N_CORES):
        b, r = i % B, i // B
        ot = res.results[i]["out"]  # [65, 1024]
        o = ot[0:D, :] / ot[D : D + 1, :]
        for j in range(N_SLOTS):
            t0 = QTILE * (2 * j + r)
            y[b, t0 : t0 + QTILE, :] = o[:, j * QTILE : (j + 1) * QTILE].T
    return y
